# revision 1
# baseline (speedup 1.0000x reference)
"""KLDiscretLoss joints kernel for TRN2 (8 NeuronCores, Bass/Tile).

Math: for each row (b,j,d) of BINS logits,
  kl_row_sum = sum_bins labels*(log_labels - log_scores)
             = w/St + log(So) - log(St)
  where St = sum(exp(t)), So = sum(exp(o)), w = sum(exp(t)*(t-o)).
(no max-subtraction needed: randn inputs, |x| <~ 6, exp is safe in f32)

Sharding: data-parallel over batch, 32 batches/core -> 1088 rows/core,
tiled as 8x[128,2048] + 1x[64,2048]. Device streams both tensors once
(memory-bound) and emits per-row partial stats; host does the final
per-row combine + batch-mean + sum-over-d + min-over-j in float64.

Schedule notes (from TimelineSim cost model):
- exp on ACT (with fused accum_out row-sum), t-o on Pool/GpSimd,
  mul+reduce on DVE: every engine stays below the ~50us DMA roofline.
- fused tensor_tensor_reduce would save a DVE pass but crashes the NEFF
  on this HW path (NRT_EXEC_UNIT_UNRECOVERABLE) -> mul + reduce.
- the last tiles are bin-halved so the post-last-load dependency chain
  (sub -> mul -> reduce) is half as long; trims ~3us off the tail.
"""

import numpy as np

import concourse.bass as bass
import concourse.tile as tile
from concourse import bacc, mybir
from concourse.bass_utils import run_bass_kernel_spmd

B, J, D, BINS = 256, 17, 2, 2048
NCORES = 8
BS = B // NCORES               # 32 batches per core
ROWS = BS * J * D              # 1088 rows per core
P = 128
NTILES = (ROWS + P - 1) // P   # 9 tiles (8 full + 1 of 64 rows)
SPLIT = {5: 2, 6: 2, 7: 2, 8: 2}  # tail tiles computed in bin-halves
NCOLS = sum(3 * SPLIT.get(i, 1) for i in range(NTILES))
F32 = mybir.dt.float32
Exp = mybir.ActivationFunctionType.Exp
Alu = mybir.AluOpType

_cache = {}


def _build_nc():
    nc = bacc.Bacc(
        "TRN2", target_bir_lowering=False, debug=False, num_devices=NCORES
    )
    o_ap = nc.dram_tensor("o_in", [ROWS, BINS], F32, kind="ExternalInput").ap()
    t_ap = nc.dram_tensor("t_in", [ROWS, BINS], F32, kind="ExternalInput").ap()
    s_ap = nc.dram_tensor("stats", [P, NCOLS], F32, kind="ExternalOutput").ap()

    with tile.TileContext(nc) as tc:
        with (
            tc.tile_pool(name="io", bufs=3) as io,
            tc.tile_pool(name="work", bufs=2) as work,
            tc.tile_pool(name="single", bufs=1) as single,
        ):
            big = single.tile([P, NCOLS], F32)
            nc.vector.memset(big[:], 0.0)
            col = 0
            for i in range(NTILES):
                r0 = i * P
                R = min(P, ROWS - r0)
                nchunk = SPLIT.get(i, 1)
                CS = BINS // nchunk
                for h in range(nchunk):
                    sl = slice(h * CS, (h + 1) * CS)
                    t_t = io.tile([P, BINS], F32, tag="t_t")
                    nc.sync.dma_start(t_t[:R, :CS], t_ap[r0 : r0 + R, sl])
                    o_t = io.tile([P, BINS], F32, tag="o_t")
                    nc.sync.dma_start(o_t[:R, :CS], o_ap[r0 : r0 + R, sl])
                    et = work.tile([P, BINS], F32, tag="et")
                    nc.scalar.activation(
                        et[:R, :CS], t_t[:R, :CS], Exp,
                        accum_out=big[:R, col : col + 1],
                    )
                    eo = work.tile([P, BINS], F32, tag="eo")
                    nc.scalar.activation(
                        eo[:R, :CS], o_t[:R, :CS], Exp,
                        accum_out=big[:R, col + 1 : col + 2],
                    )
                    diff = work.tile([P, BINS], F32, tag="diff")
                    nc.gpsimd.tensor_sub(diff[:R, :CS], t_t[:R, :CS], o_t[:R, :CS])
                    prod = work.tile([P, BINS], F32, tag="prod")
                    nc.vector.tensor_mul(prod[:R, :CS], et[:R, :CS], diff[:R, :CS])
                    nc.vector.tensor_reduce(
                        big[:R, col + 2 : col + 3], prod[:R, :CS],
                        mybir.AxisListType.X, Alu.add,
                    )
                    col += 3
            nc.sync.dma_start(s_ap[:, :], big[:, :])
    nc.compile()
    return nc


def kernel(output, target):
    output = np.ascontiguousarray(output, dtype=np.float32)
    target = np.ascontiguousarray(target, dtype=np.float32)
    assert output.shape == (B, J, D, BINS) and target.shape == (B, J, D, BINS)

    if "nc" not in _cache:
        _cache["nc"] = _build_nc()
    nc = _cache["nc"]

    in_maps = []
    for c in range(NCORES):
        sl = slice(c * BS, (c + 1) * BS)
        in_maps.append(
            {
                "o_in": output[sl].reshape(ROWS, BINS),
                "t_in": target[sl].reshape(ROWS, BINS),
            }
        )

    res = run_bass_kernel_spmd(nc, in_maps, list(range(NCORES)))
    _cache["last_results"] = res

    # host-side decode + final reduction (float64)
    per_row = np.empty((NCORES, ROWS), dtype=np.float64)
    for c in range(NCORES):
        st = res.results[c]["stats"].astype(np.float64)  # [P, NCOLS]
        St = np.zeros((NTILES, P))
        So = np.zeros((NTILES, P))
        w = np.zeros((NTILES, P))
        col = 0
        for i in range(NTILES):
            for _h in range(SPLIT.get(i, 1)):
                St[i] += st[:, col]
                So[i] += st[:, col + 1]
                w[i] += st[:, col + 2]
                col += 3
        St = St.reshape(-1)[:ROWS]
        So = So.reshape(-1)[:ROWS]
        w = w.reshape(-1)[:ROWS]
        per_row[c] = w / St + np.log(So) - np.log(St)

    per_row = per_row.reshape(B, J * D) / BINS          # per_bd, mean over bins
    per_jd = per_row.mean(axis=0)                        # [J*D]
    loss = per_jd.reshape(J, D).sum(axis=1)              # [J]
    return np.float32(loss.min())



# revision 3
# speedup vs baseline: 1.3322x; 1.3322x over previous
"""KLDiscretLoss joints kernel for TRN2 (8 NeuronCores, Bass/Tile). v2.1.

Math per row (b,j,d) of BINS logits (t = target, o = output/pred):
  kl_row_sum = w/St + log(So) - log(St)
  where St = sum(exp(t)), So = sum(exp(o)), w = sum(exp(t)*(t-o)).

v2.1 strategy (TimelineSim-guided, HW-legal ops only):
- Inputs cast to bf16 on host -> DMA bytes halve (~26us/core vs ~52 f32).
- ACT: exp(t)+St accum on all 9 tiles, exp(o)+So on 5 tiles (~33us).
- DVE: Schraudolph exp(o) for the other 4 o-tiles in 16-bit 4x-mode
  tensor_scalar (i16 codes = o*A + B; bitcast bf16; 4x copy pass with
  free accum -> So), prod = et*diff (2x tt), w accum via 4x copy pass.
- GPSIMD: diff = t-o (tensor_tensor) on 7 tiles; DVE the other 2.
Row-stats go out as f32 accum columns; host does the final combine
(w/St + log So - log St), batch-mean, sum over d, min over j in f64.

Notes pinned by micro-tests on this container:
- scalar_tensor_tensor / tensor_scalar+accum are ILLEGAL on GPSIMD
  (neuronxcc rejects TensorScalarPtr/accum on Pool).
- tensor_tensor_reduce on DVE wedges the device (NRT_EXEC_UNIT_...).
- DVE float->int16 conversion is round-to-nearest (C calibrated so).
- ACT accum sums at f32 precision (pre-rounding).
"""

import numpy as np

import concourse.bass as bass
import concourse.tile as tile
from concourse import bacc, mybir
from concourse.bass_utils import run_bass_kernel_spmd

try:
    import ml_dtypes

    _BF16 = np.dtype(ml_dtypes.bfloat16)
except Exception:  # pragma: no cover
    _BF16 = None

B, J, D, BINS = 256, 17, 2, 2048
NCORES = 8
BS = B // NCORES               # 32 batches per core
ROWS = BS * J * D              # 1088 rows per core
P = 128
NTILES = (ROWS + P - 1) // P   # 9 tiles (8 full + 1 of 64 rows)

# Schraudolph constants for bf16 (i16 = o*A + B, bitcast to bf16).
SCH_A = 2.0**7 / np.log(2.0)
SCH_C = 7.3608  # calibrated for round-to-nearest conversion, o ~ N(0,1)
SCH_B = 127.0 * 2.0**7 - SCH_C

ACT_O_TILES = (0, 2, 4, 6, 8)  # exp(o) on ACT for these tiles
GPS_DIFF_TILES = (0, 1, 2, 3, 4, 5, 6)  # diff on GPSIMD for these
F32 = mybir.dt.float32
BF16 = mybir.dt.bfloat16
I16 = mybir.dt.int16
Exp = mybir.ActivationFunctionType.Exp
Alu = mybir.AluOpType

_cache = {}


def _to_bf16(x: np.ndarray) -> np.ndarray:
    if _BF16 is not None:
        return np.ascontiguousarray(x.astype(_BF16))
    # round-to-nearest-even bit trick fallback (device sees raw bf16 bits)
    u = np.ascontiguousarray(x, dtype=np.float32).view(np.uint32)
    r = ((u >> 16) & 1).astype(np.uint32)
    return ((u + 0x7FFF + r) >> 16).astype(np.uint16)


def _build_nc():
    nc = bacc.Bacc(
        "TRN2", target_bir_lowering=False, debug=False, num_devices=NCORES
    )
    o_ap = nc.dram_tensor("o_in", [ROWS, BINS], BF16, kind="ExternalInput").ap()
    t_ap = nc.dram_tensor("t_in", [ROWS, BINS], BF16, kind="ExternalInput").ap()
    # ACT-written stats: cols 0..8 = St tile i; cols 9..13 = So of ACT_O_TILES
    sa_ap = nc.dram_tensor("stat_a", [P, 14], F32, kind="ExternalOutput").ap()
    # DVE-written stats: cols 0..3 = So of DVE o-tiles; cols 4..12 = w tile i
    sd_ap = nc.dram_tensor("stat_d", [P, 13], F32, kind="ExternalOutput").ap()

    act_so_col = {t: 9 + k for k, t in enumerate(ACT_O_TILES)}
    dve_so_col = {t: k for k, t in enumerate(
        [i for i in range(NTILES) if i not in ACT_O_TILES])}

    with tile.TileContext(nc) as tc:
        with (
            tc.tile_pool(name="io", bufs=3) as io,
            tc.tile_pool(name="work", bufs=2) as work,
            tc.tile_pool(name="single", bufs=1) as single,
        ):
            st_a = single.tile([P, 14], F32, name="st_a")
            st_d = single.tile([P, 13], F32, name="st_d")
            nc.scalar.memzero(st_a[:])
            nc.vector.memset(st_d[:], 0.0)
            for i in range(NTILES):
                r0 = i * P
                R = min(P, ROWS - r0)

                t_t = io.tile([P, BINS], BF16, name=f"t_{i}", tag="t_t")
                nc.sync.dma_start(t_t[:R, :], t_ap[r0 : r0 + R, :])
                o_t = io.tile([P, BINS], BF16, name=f"o_{i}", tag="o_t")
                nc.sync.dma_start(o_t[:R, :], o_ap[r0 : r0 + R, :])

                # exp(t) + St on ACT always
                et = work.tile([P, BINS], BF16, name=f"et_{i}", tag="et")
                nc.scalar.activation(
                    et[:R, :], t_t[:R, :], Exp, accum_out=st_a[:R, i : i + 1]
                )

                # So path
                if i in ACT_O_TILES:
                    eo = work.tile([P, BINS], BF16, name=f"eo_{i}", tag="eo")
                    nc.scalar.activation(
                        eo[:R, :], o_t[:R, :], Exp,
                        accum_out=st_a[:R, act_so_col[i] : act_so_col[i] + 1],
                    )
                else:
                    c = dve_so_col[i]
                    esch = work.tile([P, BINS], I16, name=f"es_{i}", tag="esch")
                    nc.vector.tensor_scalar(
                        esch[:R, :], o_t[:R, :], SCH_A, SCH_B, Alu.mult, Alu.add
                    )
                    scp = work.tile([P, BINS], BF16, name=f"sc_{i}", tag="socp")
                    nc.vector.tensor_scalar(
                        scp[:R, :], esch[:R, :].bitcast(BF16), 1.0, 0.0,
                        Alu.mult, Alu.add, accum_out=st_d[:R, c : c + 1],
                    )

                # diff
                diff = work.tile([P, BINS], BF16, name=f"df_{i}", tag="diff")
                eng = nc.gpsimd if i in GPS_DIFF_TILES else nc.vector
                eng.tensor_sub(diff[:R, :], t_t[:R, :], o_t[:R, :])

                # prod + w accum on DVE
                prod = work.tile([P, BINS], BF16, name=f"pr_{i}", tag="prod")
                nc.vector.tensor_mul(prod[:R, :], et[:R, :], diff[:R, :])
                wscr = work.tile([P, BINS], BF16, name=f"ws_{i}", tag="wscr")
                nc.vector.tensor_scalar(
                    wscr[:R, :], prod[:R, :], 1.0, 0.0, Alu.mult, Alu.add,
                    accum_out=st_d[:R, 4 + i : 5 + i],
                )
            nc.sync.dma_start(sa_ap[:, :], st_a[:, :])
            nc.sync.dma_start(sd_ap[:, :], st_d[:, :])
    nc.compile()
    return nc


def kernel(output, target):
    output = np.ascontiguousarray(output, dtype=np.float32)
    target = np.ascontiguousarray(target, dtype=np.float32)
    assert output.shape == (B, J, D, BINS) and target.shape == (B, J, D, BINS)

    if "nc" not in _cache:
        _cache["nc"] = _build_nc()
    nc = _cache["nc"]

    o16 = _to_bf16(output).reshape(B, J * D, BINS)
    t16 = _to_bf16(target).reshape(B, J * D, BINS)

    in_maps = []
    for c in range(NCORES):
        sl = slice(c * BS, (c + 1) * BS)
        in_maps.append(
            {
                "o_in": o16[sl].reshape(ROWS, BINS),
                "t_in": t16[sl].reshape(ROWS, BINS),
            }
        )

    res = run_bass_kernel_spmd(nc, in_maps, list(range(NCORES)))
    _cache["last_results"] = res

    act_so_col = {t: 9 + k for k, t in enumerate(ACT_O_TILES)}
    dve_so_col = {t: k for k, t in enumerate(
        [i for i in range(NTILES) if i not in ACT_O_TILES])}

    per_row = np.empty((NCORES, ROWS), dtype=np.float64)
    for c in range(NCORES):
        sa = res.results[c]["stat_a"].astype(np.float64)  # [P, 14]
        sd = res.results[c]["stat_d"].astype(np.float64)  # [P, 13]
        St = np.empty((NTILES, P))
        So = np.empty((NTILES, P))
        w = np.empty((NTILES, P))
        for i in range(NTILES):
            St[i] = sa[:, i]
            So[i] = (
                sa[:, act_so_col[i]] if i in ACT_O_TILES
                else sd[:, dve_so_col[i]]
            )
            w[i] = sd[:, 4 + i]
        St = St.reshape(-1)[:ROWS]
        So = So.reshape(-1)[:ROWS]
        w = w.reshape(-1)[:ROWS]
        per_row[c] = w / St + np.log(So) - np.log(St)

    per_row = per_row.reshape(B, J * D) / BINS          # per_bd, mean over bins
    per_jd = per_row.mean(axis=0)                        # [J*D]
    loss = per_jd.reshape(J, D).sum(axis=1)              # [J]
    return np.float32(loss.min())


# revision 11
# speedup vs baseline: 1.6593x; 1.2455x over previous
"""KLDiscretLoss joints kernel for TRN2 (8 NeuronCores, Bass/Tile). v3.

Math per row (b,j,d) of BINS logits (t = target, o = output/pred):
  kl_row_sum = w/St + log(So) - log(St)
  where St = sum(exp(t)), So = sum(exp(o)), w = sum(exp(t)*(t-o)).

Strategy (TimelineSim-guided, HW-legal ops only):
- Inputs cast to bf16 on host -> DMA bytes halve (~25us/core).
- ACT: exp(t)+St accum on all tiles, exp(o)+So on ACT_O_TILES.
- DVE: Schraudolph exp(o) for other tiles (16-bit 4x tensor_scalar:
  i16 codes = o*A + B, bitcast bf16, 4x copy pass w/ free accum -> So);
  prod = et*diff (2x tt); w accum via 4x copy pass; diff for non-GPS tiles.
- GPSIMD: diff = t-o on GPS_DIFF_TILES (early tiles).
- Tile order puts the 64-row tile first (half-size DMA -> fast start);
  first tile's exp(t) and last tile's whole chain are column-split to
  shorten pipeline fill/drain.
Host combines stats in f64: w/St + log So - log St, batch-mean, min.

Pinned by micro-tests: scalar_tensor_tensor & tensor_scalar+accum are
illegal on GPSIMD; tensor_tensor_reduce wedges the device; DVE f32->i16
conversion rounds to nearest; ACT/DVE accum_out sums at f32 precision.
"""

import numpy as np

import concourse.bass as bass
import concourse.tile as tile
from concourse import bacc, mybir
from concourse.bass_utils import run_bass_kernel_spmd

try:
    import ml_dtypes

    _BF16 = np.dtype(ml_dtypes.bfloat16)
except Exception:  # pragma: no cover
    _BF16 = None

B, J, D, BINS = 256, 17, 2, 2048
NCORES = 8
BS = B // NCORES               # 32 batches per core
ROWS = BS * J * D              # 1088 rows per core
P = 128
NTILES = (ROWS + P - 1) // P   # 9 tiles (8 full + 1 of 64 rows)

SCH_A = 2.0**7 / np.log(2.0)
SCH_C = 7.3608  # calibrated for round-to-nearest conversion, o ~ N(0,1)
SCH_B = 127.0 * 2.0**7 - SCH_C

# --- schedule knobs (tile ids are DATA tile ids 0..8; 8 is the 64-row tile)
TILE_ORDER = (8, 0, 1, 2, 3, 4, 5, 6, 7)
ACT_O_TILES = (1, 3, 5, 7)     # exp(o) on ACT for these data tiles
GPS_DIFF_TILES = (0, 2, 4)     # diff on GPSIMD for these data tiles
LAG = 2                        # software-pipeline lag (in order positions)
GPS_BACK_LAG = 5               # extra-late prod/wacc for GPS-diff tiles
SPLIT_FIRST = True             # col-split first tile's t-load + exp(t)
SPLIT_LAST = True              # col-split last tile's whole chain

F32 = mybir.dt.float32
BF16 = mybir.dt.bfloat16
I16 = mybir.dt.int16
Exp = mybir.ActivationFunctionType.Exp
Alu = mybir.AluOpType

_cache = {}


def _to_bf16(x: np.ndarray) -> np.ndarray:
    if _BF16 is not None:
        return np.ascontiguousarray(x.astype(_BF16))
    u = np.ascontiguousarray(x, dtype=np.float32).view(np.uint32)
    r = ((u >> 16) & 1).astype(np.uint32)
    return ((u + 0x7FFF + r) >> 16).astype(np.uint16)


class _Cols:
    """Column allocator for an accum stats tile: stat key -> list of cols."""

    def __init__(self):
        self.map = {}
        self.n = 0

    def col(self, key):
        self.map.setdefault(key, []).append(self.n)
        self.n += 1
        return self.n - 1


def _schedule():
    """Return the plan: per order position, the ops to emit."""
    ca, cd = _Cols(), _Cols()  # ACT-written / DVE-written stat cols
    return ca, cd


def _build_nc():
    nc = bacc.Bacc(
        "TRN2", target_bir_lowering=False, debug=False, num_devices=NCORES
    )
    o_ap = nc.dram_tensor("o_in", [ROWS, BINS], BF16, kind="ExternalInput").ap()
    t_ap = nc.dram_tensor("t_in", [ROWS, BINS], BF16, kind="ExternalInput").ap()

    ca, cd = _Cols(), _Cols()
    # pre-allocate columns in a fixed traversal so host decode can re-derive
    plan = {}
    first_i = TILE_ORDER[0]
    last_i = TILE_ORDER[-1]
    for i in TILE_ORDER:
        p = {}
        p["st_cols"] = (
            [ca.col(("St", i)), ca.col(("St", i))]
            if (SPLIT_FIRST and i == first_i) or (SPLIT_LAST and i == last_i)
            else [ca.col(("St", i))]
        )
        if i in ACT_O_TILES:
            p["so_act"] = ca.col(("So", i))
        else:
            p["so_dve"] = cd.col(("So", i))
        p["w_cols"] = (
            [cd.col(("w", i)), cd.col(("w", i))]
            if (SPLIT_LAST and i == last_i)
            else [cd.col(("w", i))]
        )
        plan[i] = p

    NA, ND = ca.n, cd.n
    sa_ap = nc.dram_tensor("stat_a", [P, NA], F32, kind="ExternalOutput").ap()
    sd_ap = nc.dram_tensor("stat_d", [P, ND], F32, kind="ExternalOutput").ap()

    H = BINS // 2
    with tile.TileContext(nc) as tc:
        with (
            tc.tile_pool(name="io", bufs=3) as io,
            tc.tile_pool(name="work", bufs=2) as work,
            tc.tile_pool(name="single", bufs=1) as single,
        ):
            st_a = single.tile([P, NA], F32, name="st_a")
            st_d = single.tile([P, ND], F32, name="st_d")
            nc.scalar.memzero(st_a[:])
            nc.vector.memset(st_d[:], 0.0)

            ets, diffs, rr = {}, {}, {}

            def front(i):
                r0 = i * P
                R = min(P, ROWS - r0)
                rr[i] = R
                p = plan[i]
                split_t = SPLIT_FIRST and i == first_i
                split_all = SPLIT_LAST and i == last_i

                nbuf = max(LAG, GPS_BACK_LAG if GPS_DIFF_TILES else LAG) + 2
                t_t = io.tile([P, BINS], BF16, name=f"t_{i}", tag="t_t",
                              bufs=nbuf)
                if split_t or split_all:
                    nc.sync.dma_start(t_t[:R, :H], t_ap[r0 : r0 + R, :H])
                    nc.sync.dma_start(t_t[:R, H:], t_ap[r0 : r0 + R, H:])
                else:
                    nc.sync.dma_start(t_t[:R, :], t_ap[r0 : r0 + R, :])
                o_t = io.tile([P, BINS], BF16, name=f"o_{i}", tag="o_t",
                              bufs=nbuf)
                nc.sync.dma_start(o_t[:R, :], o_ap[r0 : r0 + R, :])

                et = work.tile([P, BINS], BF16, name=f"et_{i}", tag="et",
                               bufs=nbuf)
                if split_t or split_all:
                    c0, c1 = p["st_cols"]
                    nc.scalar.activation(et[:R, :H], t_t[:R, :H], Exp,
                                         accum_out=st_a[:R, c0 : c0 + 1])
                    nc.scalar.activation(et[:R, H:], t_t[:R, H:], Exp,
                                         accum_out=st_a[:R, c1 : c1 + 1])
                else:
                    c0 = p["st_cols"][0]
                    nc.scalar.activation(et[:R, :], t_t[:R, :], Exp,
                                         accum_out=st_a[:R, c0 : c0 + 1])
                ets[i] = et

                if i in ACT_O_TILES:
                    c = p["so_act"]
                    eo = work.tile([P, BINS], BF16, name=f"eo_{i}", tag="eo")
                    nc.scalar.activation(eo[:R, :], o_t[:R, :], Exp,
                                         accum_out=st_a[:R, c : c + 1])
                else:
                    c = p["so_dve"]
                    esch = work.tile([P, BINS], I16, name=f"es_{i}", tag="esch")
                    nc.vector.tensor_scalar(
                        esch[:R, :], o_t[:R, :], SCH_A, SCH_B, Alu.mult, Alu.add
                    )
                    scp = work.tile([P, BINS], BF16, name=f"sc_{i}", tag="socp")
                    nc.vector.tensor_scalar(
                        scp[:R, :], esch[:R, :].bitcast(BF16), 1.0, 0.0,
                        Alu.mult, Alu.add, accum_out=st_d[:R, c : c + 1],
                    )

                diff = work.tile([P, BINS], BF16, name=f"df_{i}", tag="diff",
                                 bufs=nbuf)
                eng = nc.gpsimd if i in GPS_DIFF_TILES else nc.vector
                if split_all:
                    eng.tensor_sub(diff[:R, :H], t_t[:R, :H], o_t[:R, :H])
                    eng.tensor_sub(diff[:R, H:], t_t[:R, H:], o_t[:R, H:])
                else:
                    eng.tensor_sub(diff[:R, :], t_t[:R, :], o_t[:R, :])
                diffs[i] = diff

            def back(i):
                R = rr[i]
                p = plan[i]
                split_all = SPLIT_LAST and i == last_i
                halves = (
                    [(slice(0, H), p["w_cols"][0]), (slice(H, BINS), p["w_cols"][1])]
                    if split_all
                    else [(slice(0, BINS), p["w_cols"][0])]
                )
                for k, (sl, wc) in enumerate(halves):
                    prod = work.tile([P, BINS], BF16, name=f"pr_{i}_{k}",
                                     tag="prod")
                    nc.vector.tensor_mul(
                        prod[:R, sl], ets[i][:R, sl], diffs[i][:R, sl]
                    )
                    wscr = work.tile([P, BINS], BF16, name=f"ws_{i}_{k}",
                                     tag="wscr")
                    nc.vector.tensor_scalar(
                        wscr[:R, sl], prod[:R, sl], 1.0, 0.0, Alu.mult, Alu.add,
                        accum_out=st_d[:R, wc : wc + 1],
                    )

            npos = len(TILE_ORDER)
            due = {}
            for pos, i in enumerate(TILE_ORDER):
                lag = GPS_BACK_LAG if i in GPS_DIFF_TILES else LAG
                due.setdefault(min(pos + lag, npos + LAG - 1), []).append(i)
            for pos in range(npos + LAG):
                if pos < npos:
                    front(TILE_ORDER[pos])
                for i in due.get(pos, []):
                    back(i)

            nc.sync.dma_start(sa_ap[:, :], st_a[:, :])
            nc.sync.dma_start(sd_ap[:, :], st_d[:, :])
    nc.compile()
    _cache["plan"] = (plan, NA, ND)
    return nc


def kernel(output, target):
    output = np.ascontiguousarray(output, dtype=np.float32)
    target = np.ascontiguousarray(target, dtype=np.float32)
    assert output.shape == (B, J, D, BINS) and target.shape == (B, J, D, BINS)

    if "nc" not in _cache:
        _cache["nc"] = _build_nc()
    nc = _cache["nc"]
    plan, NA, ND = _cache["plan"]

    o16 = _to_bf16(output).reshape(B, J * D, BINS)
    t16 = _to_bf16(target).reshape(B, J * D, BINS)

    in_maps = []
    for c in range(NCORES):
        sl = slice(c * BS, (c + 1) * BS)
        in_maps.append(
            {
                "o_in": o16[sl].reshape(ROWS, BINS),
                "t_in": t16[sl].reshape(ROWS, BINS),
            }
        )

    res = run_bass_kernel_spmd(nc, in_maps, list(range(NCORES)))
    _cache["last_results"] = res

    per_row = np.empty((NCORES, ROWS), dtype=np.float64)
    for c in range(NCORES):
        sa = res.results[c]["stat_a"].astype(np.float64)
        sd = res.results[c]["stat_d"].astype(np.float64)
        St = np.empty((NTILES, P))
        So = np.empty((NTILES, P))
        w = np.empty((NTILES, P))
        for i in range(NTILES):
            p = plan[i]
            St[i] = sum(sa[:, cc] for cc in p["st_cols"])
            So[i] = sa[:, p["so_act"]] if "so_act" in p else sd[:, p["so_dve"]]
            w[i] = sum(sd[:, cc] for cc in p["w_cols"])
        St = St.reshape(-1)[:ROWS]
        So = So.reshape(-1)[:ROWS]
        w = w.reshape(-1)[:ROWS]
        per_row[c] = w / St + np.log(So) - np.log(St)

    per_row = per_row.reshape(B, J * D) / BINS          # per_bd, mean over bins
    per_jd = per_row.mean(axis=0)                        # [J*D]
    loss = per_jd.reshape(J, D).sum(axis=1)              # [J]
    return np.float32(loss.min())


# revision 15
# speedup vs baseline: 1.6657x; 1.0039x over previous
"""KLDiscretLoss joints kernel for TRN2 (8 NeuronCores, Bass/Tile). v4.

Math per row (b,j,d) of BINS logits (t = target, o = output/pred):
  kl_row_sum = w/St + log(So) - log(St)
  where St = sum(exp(t)), So = sum(exp(o)), w = sum(exp(t)*(t-o)).

Strategy (TimelineSim-guided, HW-legal ops only):
- Inputs cast to bf16 on host -> DMA bytes halve (~25us/core).
- ACT: exp(t)+St accum on all tiles, exp(o)+So on ACT_O_TILES.
- DVE: Schraudolph exp(o) for other tiles (16-bit 4x tensor_scalar:
  i16 codes = o*A + B, bitcast bf16, 4x copy pass w/ free accum -> So);
  prod = et*diff (2x tt); w accum via 4x copy pass; diff for non-GPS tiles.
- GPSIMD: diff = t-o on GPS_DIFF_TILES; their prod/wacc run extra late.
- t-loads run O_LAG tiles ahead of o-loads so ACT's exp(t) stream never
  starves; 64-row tile goes first; first exp(t) and the last tile's
  chain are column-split to shorten fill/drain.
Host combines stats in f64: w/St + log So - log St, batch-mean, min.

Pinned by micro-tests: scalar_tensor_tensor & tensor_scalar+accum are
illegal on GPSIMD; tensor_tensor_reduce wedges the device; DVE f32->i16
conversion rounds to nearest; ACT/DVE accum_out sums at f32 precision.
"""

import numpy as np

import concourse.bass as bass
import concourse.tile as tile
from concourse import bacc, mybir
from concourse.bass_utils import run_bass_kernel_spmd

try:
    import ml_dtypes

    _BF16 = np.dtype(ml_dtypes.bfloat16)
except Exception:  # pragma: no cover
    _BF16 = None

B, J, D, BINS = 256, 17, 2, 2048
NCORES = 8
BS = B // NCORES               # 32 batches per core
ROWS = BS * J * D              # 1088 rows per core
P = 128
NTILES = (ROWS + P - 1) // P   # 9 tiles (8 full + 1 of 64 rows)

SCH_A = 2.0**7 / np.log(2.0)
SCH_C = 7.3608  # calibrated for round-to-nearest conversion, o ~ N(0,1)
SCH_B = 127.0 * 2.0**7 - SCH_C

# --- schedule knobs (tile ids are DATA tile ids 0..8; 8 is the 64-row tile)
TILE_ORDER = (8, 0, 1, 2, 3, 4, 5, 6, 7)
ACT_O_TILES = (1, 3, 5, 7)     # exp(o) on ACT for these data tiles
GPS_DIFF_TILES = (0, 2, 4)     # diff on GPSIMD for these data tiles
O_LAG = 0                      # o-side work lags t-side by this many slots
LAG = 2                        # prod/wacc lag after the o-side work
GPS_BACK_LAG = 4               # prod/wacc lag for GPS-diff tiles
SPLIT_FIRST = True             # col-split first tile's t-load + exp(t)
SPLIT_LAST = True              # col-split last tile's whole chain
GPS_MEMSET = True              # zero the stat tiles on GPSIMD
SPLIT_O_LOAD = True            # also split the last tile's o DMA
DMA_T_AHEAD = False            # t-loads run one tile ahead in the DMA queue

F32 = mybir.dt.float32
BF16 = mybir.dt.bfloat16
I16 = mybir.dt.int16
Exp = mybir.ActivationFunctionType.Exp
Alu = mybir.AluOpType

_cache = {}


def _to_bf16(x: np.ndarray) -> np.ndarray:
    if _BF16 is not None:
        return np.ascontiguousarray(x.astype(_BF16))
    u = np.ascontiguousarray(x, dtype=np.float32).view(np.uint32)
    r = ((u >> 16) & 1).astype(np.uint32)
    return ((u + 0x7FFF + r) >> 16).astype(np.uint16)


class _Cols:
    def __init__(self):
        self.n = 0

    def col(self):
        self.n += 1
        return self.n - 1


def _make_plan():
    ca, cd = _Cols(), _Cols()
    plan = {}
    first_i = TILE_ORDER[0]
    last_i = TILE_ORDER[-1]
    for i in TILE_ORDER:
        p = {}
        split = (SPLIT_FIRST and i == first_i) or (SPLIT_LAST and i == last_i)
        p["st_cols"] = [ca.col(), ca.col()] if split else [ca.col()]
        if i in ACT_O_TILES:
            p["so_act"] = ca.col()
        else:
            p["so_dve"] = cd.col()
        p["w_cols"] = (
            [cd.col(), cd.col()] if (SPLIT_LAST and i == last_i) else [cd.col()]
        )
        plan[i] = p
    return plan, ca.n, cd.n


def _build_nc():
    nc = bacc.Bacc(
        "TRN2", target_bir_lowering=False, debug=False, num_devices=NCORES
    )
    o_ap = nc.dram_tensor("o_in", [ROWS, BINS], BF16, kind="ExternalInput").ap()
    t_ap = nc.dram_tensor("t_in", [ROWS, BINS], BF16, kind="ExternalInput").ap()

    plan, NA, ND = _make_plan()
    sa_ap = nc.dram_tensor("stat_a", [P, NA], F32, kind="ExternalOutput").ap()
    sd_ap = nc.dram_tensor("stat_d", [P, ND], F32, kind="ExternalOutput").ap()

    first_i = TILE_ORDER[0]
    last_i = TILE_ORDER[-1]
    H = BINS // 2
    nbuf = max(LAG, GPS_BACK_LAG if GPS_DIFF_TILES else LAG) + O_LAG + 2

    with tile.TileContext(nc) as tc:
        with (
            tc.tile_pool(name="io", bufs=3) as io,
            tc.tile_pool(name="work", bufs=2) as work,
            tc.tile_pool(name="single", bufs=1) as single,
        ):
            st_a = single.tile([P, NA], F32, name="st_a")
            st_d = single.tile([P, ND], F32, name="st_d")
            if GPS_MEMSET:
                nc.gpsimd.memset(st_a[:], 0.0)
                nc.gpsimd.memset(st_d[:], 0.0)
            else:
                nc.scalar.memzero(st_a[:])
                nc.vector.memset(st_d[:], 0.0)

            tts, ots, ets, diffs, rr = {}, {}, {}, {}, {}

            def dma_t(i):
                r0 = i * P
                R = min(P, ROWS - r0)
                rr[i] = R
                split = (SPLIT_FIRST and i == first_i) or (
                    SPLIT_LAST and i == last_i
                )
                t_t = io.tile([P, BINS], BF16, name=f"t_{i}", tag="t_t",
                              bufs=nbuf)
                if split:
                    nc.sync.dma_start(t_t[:R, :H], t_ap[r0 : r0 + R, :H])
                    nc.sync.dma_start(t_t[:R, H:], t_ap[r0 : r0 + R, H:])
                else:
                    nc.sync.dma_start(t_t[:R, :], t_ap[r0 : r0 + R, :])
                tts[i] = t_t

            def t_stage(i):
                R = rr[i]
                p = plan[i]
                split = (SPLIT_FIRST and i == first_i) or (
                    SPLIT_LAST and i == last_i
                )
                t_t = tts[i]
                et = work.tile([P, BINS], BF16, name=f"et_{i}", tag="et",
                               bufs=nbuf)
                if split:
                    c0, c1 = p["st_cols"]
                    nc.scalar.activation(et[:R, :H], t_t[:R, :H], Exp,
                                         accum_out=st_a[:R, c0 : c0 + 1])
                    nc.scalar.activation(et[:R, H:], t_t[:R, H:], Exp,
                                         accum_out=st_a[:R, c1 : c1 + 1])
                else:
                    c0 = p["st_cols"][0]
                    nc.scalar.activation(et[:R, :], t_t[:R, :], Exp,
                                         accum_out=st_a[:R, c0 : c0 + 1])
                ets[i] = et

            def o_stage(i):
                R = rr[i]
                r0 = i * P
                p = plan[i]
                split = SPLIT_LAST and i == last_i
                o_t = io.tile([P, BINS], BF16, name=f"o_{i}", tag="o_t",
                              bufs=nbuf)
                if split and SPLIT_O_LOAD:
                    nc.sync.dma_start(o_t[:R, :H], o_ap[r0 : r0 + R, :H])
                    nc.sync.dma_start(o_t[:R, H:], o_ap[r0 : r0 + R, H:])
                else:
                    nc.sync.dma_start(o_t[:R, :], o_ap[r0 : r0 + R, :])
                ots[i] = o_t

                if i in ACT_O_TILES:
                    c = p["so_act"]
                    eo = work.tile([P, BINS], BF16, name=f"eo_{i}", tag="eo")
                    nc.scalar.activation(eo[:R, :], o_t[:R, :], Exp,
                                         accum_out=st_a[:R, c : c + 1])
                else:
                    c = p["so_dve"]
                    esch = work.tile([P, BINS], I16, name=f"es_{i}", tag="esch")
                    nc.vector.tensor_scalar(
                        esch[:R, :], o_t[:R, :], SCH_A, SCH_B, Alu.mult, Alu.add
                    )
                    scp = work.tile([P, BINS], BF16, name=f"sc_{i}", tag="socp")
                    nc.vector.tensor_scalar(
                        scp[:R, :], esch[:R, :].bitcast(BF16), 1.0, 0.0,
                        Alu.mult, Alu.add, accum_out=st_d[:R, c : c + 1],
                    )

                diff = work.tile([P, BINS], BF16, name=f"df_{i}", tag="diff",
                                 bufs=nbuf)
                eng = nc.gpsimd if i in GPS_DIFF_TILES else nc.vector
                if split:
                    eng.tensor_sub(diff[:R, :H], tts[i][:R, :H], o_t[:R, :H])
                    eng.tensor_sub(diff[:R, H:], tts[i][:R, H:], o_t[:R, H:])
                else:
                    eng.tensor_sub(diff[:R, :], tts[i][:R, :], o_t[:R, :])
                diffs[i] = diff

            def back(i):
                R = rr[i]
                p = plan[i]
                split = SPLIT_LAST and i == last_i
                halves = (
                    [(slice(0, H), p["w_cols"][0]),
                     (slice(H, BINS), p["w_cols"][1])]
                    if split
                    else [(slice(0, BINS), p["w_cols"][0])]
                )
                for k, (sl, wc) in enumerate(halves):
                    prod = work.tile([P, BINS], BF16, name=f"pr_{i}_{k}",
                                     tag="prod")
                    nc.vector.tensor_mul(
                        prod[:R, sl], ets[i][:R, sl], diffs[i][:R, sl]
                    )
                    wscr = work.tile([P, BINS], BF16, name=f"ws_{i}_{k}",
                                     tag="wscr")
                    nc.vector.tensor_scalar(
                        wscr[:R, sl], prod[:R, sl], 1.0, 0.0, Alu.mult, Alu.add,
                        accum_out=st_d[:R, wc : wc + 1],
                    )

            npos = len(TILE_ORDER)
            maxpos = npos + O_LAG + LAG
            due = {}
            for pos, i in enumerate(TILE_ORDER):
                lag = GPS_BACK_LAG if i in GPS_DIFF_TILES else LAG
                due.setdefault(min(pos + O_LAG + lag, maxpos - 1), []).append(i)
            if DMA_T_AHEAD:
                dma_t(TILE_ORDER[0])
            for pos in range(maxpos):
                if pos < npos:
                    if DMA_T_AHEAD:
                        if pos + 1 < npos:
                            dma_t(TILE_ORDER[pos + 1])
                    else:
                        dma_t(TILE_ORDER[pos])
                    t_stage(TILE_ORDER[pos])
                opos = pos - O_LAG
                if 0 <= opos < npos:
                    o_stage(TILE_ORDER[opos])
                for i in due.get(pos, []):
                    back(i)

            nc.sync.dma_start(sa_ap[:, :], st_a[:, :])
            nc.sync.dma_start(sd_ap[:, :], st_d[:, :])
    nc.compile()
    _cache["plan"] = (plan, NA, ND)
    return nc


def kernel(output, target):
    output = np.ascontiguousarray(output, dtype=np.float32)
    target = np.ascontiguousarray(target, dtype=np.float32)
    assert output.shape == (B, J, D, BINS) and target.shape == (B, J, D, BINS)

    if "nc" not in _cache:
        _cache["nc"] = _build_nc()
    nc = _cache["nc"]
    plan, NA, ND = _cache["plan"]

    o16 = _to_bf16(output).reshape(B, J * D, BINS)
    t16 = _to_bf16(target).reshape(B, J * D, BINS)

    in_maps = []
    for c in range(NCORES):
        sl = slice(c * BS, (c + 1) * BS)
        in_maps.append(
            {
                "o_in": o16[sl].reshape(ROWS, BINS),
                "t_in": t16[sl].reshape(ROWS, BINS),
            }
        )

    res = run_bass_kernel_spmd(nc, in_maps, list(range(NCORES)))
    _cache["last_results"] = res

    per_row = np.empty((NCORES, ROWS), dtype=np.float64)
    for c in range(NCORES):
        sa = res.results[c]["stat_a"].astype(np.float64)
        sd = res.results[c]["stat_d"].astype(np.float64)
        St = np.empty((NTILES, P))
        So = np.empty((NTILES, P))
        w = np.empty((NTILES, P))
        for i in range(NTILES):
            p = plan[i]
            St[i] = sum(sa[:, cc] for cc in p["st_cols"])
            So[i] = sa[:, p["so_act"]] if "so_act" in p else sd[:, p["so_dve"]]
            w[i] = sum(sd[:, cc] for cc in p["w_cols"])
        St = St.reshape(-1)[:ROWS]
        So = So.reshape(-1)[:ROWS]
        w = w.reshape(-1)[:ROWS]
        per_row[c] = w / St + np.log(So) - np.log(St)

    per_row = per_row.reshape(B, J * D) / BINS          # per_bd, mean over bins
    per_jd = per_row.mean(axis=0)                        # [J*D]
    loss = per_jd.reshape(J, D).sum(axis=1)              # [J]
    return np.float32(loss.min())
